# revision 16
# baseline (speedup 1.0000x reference)
"""Trainium2 Bass kernel for nn_DebugQuantizedLinear.

Computes out = x @ W_deq.T where
  W_deq = ((W_q - zeros) * scales).reshape(K, N) * mu2[:, None] * mu1[None, :]
  x: [B, N] f32, W_q: [K, N] int (values 0..15), out: [B, K] f32
  K=11008, N=4096, B=8192, group size 64 along N (NG=64 groups).

Strategy (8 NeuronCores, tensor-parallel along K):
  - K padded 11008 -> 11264 = 8 * 1408; core c owns rows [c*1408, (c+1)*1408).
  - Host supplies x transposed (xT [N, B] f32, replicated) and W_q packed as
    int8 (values 0..15, lossless) so the weight DMA is 4x smaller.
  - Phase 1 (per core, once): per half-k-tile, DMA the int8 W_q slice with an
    on-the-fly cast to fp16, dequantize in natural [k, n] layout with three
    full-width fp16 tensor_tensor ops on DVE:
       s_full = bcast(scales*mu2) * mu1_full     (mu1 folded here)
       w      = (Q - bcast(zeros)) * s_full
    then XBAR DMA-transpose ([128 k, 2048 n] -> [128 n, 16 nt, 128 k]) into
    the SBUF-resident fp16 W^T [N, 1408].  No PE transposes at all: the PE
    does nothing but the 5632 real matmuls.
  - Phase 2: stream xT in 512-column half-panels (cast f32->fp16 by DMA, 4
    chunk tiles per panel so the first matmuls start after ~2MB, not ~8MB),
    accumulate out^T tiles [128 k, 512 b] in PSUM over the 32 n-tiles,
    drain to SBUF via the scalar engine, DMA to DRAM outT [1408, B] f32.
  - Host assembles out[B, K] from the 8 outT shards (transpose + concat).

fp16 weights/activations (and fp16 zeros/scales) with fp32 PSUM accumulation
give ~5e-4 relative error vs the f32 reference.
"""

import os
from contextlib import ExitStack

import numpy as np

K, N, B = 11008, 4096, 8192
GROUP = 64
NG = N // GROUP
NCORES = 8
KC = 1408               # per-core padded K rows
KPAD = KC * NCORES      # 11264
P = 128

_PROGRAM_CACHE = {}
LAST_RESULTS = None     # BassKernelResults of the most recent run (for test.py)


def _build_program(kc=KC, b=B, bh=512):
    """Build the SPMD Bass program (identical on all cores)."""
    import concourse.bacc as bacc
    import concourse.bass as bass
    import concourse.mybir as mybir
    from concourse.tile import TileContext

    f32 = mybir.dt.float32
    f16 = mybir.dt.float16
    i16 = mybir.dt.int16

    nkt = kc // P           # 11 k-tiles per core
    nnt = N // P            # 32 n-tiles
    nh = b // bh            # 16 half-panels
    nxc = 4                 # x chunk tiles per half-panel
    cnt = nnt // nxc        # n-tiles per x chunk
    HGR = NG // 2           # 32 groups per half-k-tile
    HN = N // 2             # 2048 columns per half-k-tile
    sub = mybir.AluOpType.subtract
    mul = mybir.AluOpType.mult

    nc = bacc.Bacc(num_swdge_queues=4)
    xT = nc.declare_dram_parameter("xT", [N, b], f32, isOutput=False)
    wq = nc.declare_dram_parameter("wq", [kc, N], i16, isOutput=False)
    zr = nc.declare_dram_parameter("zr", [P, nkt * NG], f32, isOutput=False)
    sc = nc.declare_dram_parameter("sc", [P, nkt * NG], f32, isOutput=False)
    mu1 = nc.declare_dram_parameter("mu1", [1, N], f32, isOutput=False)
    mu2 = nc.declare_dram_parameter("mu2", [P, nkt], f32, isOutput=False)
    outT = nc.declare_dram_parameter("outT", [kc, b], f32, isOutput=True)

    with TileContext(nc) as tc, ExitStack() as ctx:
        const = ctx.enter_context(tc.tile_pool(name="const", bufs=1))
        mu2_t = const.tile([P, nkt], f32, name="mu2_t")
        nc.sync.dma_start(out=mu2_t[:, :], in_=mu2[:, :])
        zr_t = const.tile([P, nkt, NG], f32, name="zr_t")
        nc.sync.dma_start(out=zr_t[:, :, :], in_=zr[:, :])
        sc_t = const.tile([P, nkt, NG], f32, name="sc_t")
        nc.sync.dma_start(out=sc_t[:, :, :], in_=sc[:, :])
        # mu1 replicated across all 128 partitions, fp16, natural n order.
        # Two half-tiles so the first s_full build only waits ~1MB of DMA.
        mu1f = [const.tile([P, HN], f16, name=f"mu1f{hk}") for hk in range(2)]
        for hk in range(2):
            nc.gpsimd.dma_start(
                out=mu1f[hk][:, :],
                in_=mu1[:, hk * HN:(hk + 1) * HN].broadcast_to((P, HN)))
        # fp16 scales*mu2 (per-partition k rows); filled per k-tile in phase 1.
        sp16 = const.tile([P, nkt, NG], f16, name="sp16")

        # SBUF-resident transposed dequantized weights:
        # [128 n-partitions, n_tile, 128 k] fp16 per k-tile.
        wdqT = [const.tile([P, nnt, P], f16, name=f"wdqT_{kt}") for kt in range(nkt)]

        wqpool = ctx.enter_context(tc.tile_pool(name="wqpool", bufs=2))
        wdqpool = ctx.enter_context(tc.tile_pool(name="wdqpool", bufs=2))
        sfpool = ctx.enter_context(tc.tile_pool(name="sfpool", bufs=1))
        xpool = ctx.enter_context(tc.tile_pool(name="xpool", bufs=1))
        xspool = ctx.enter_context(tc.tile_pool(name="xspool", bufs=1))
        opsum = ctx.enter_context(tc.tile_pool(name="opsum", bufs=8, space="PSUM"))
        opool = ctx.enter_context(tc.tile_pool(name="opool", bufs=2))

        def load_x_half(h):
            # Raw f32 x chunks on the two fast HWDGE queues (split sync/ACT),
            # engine-cast to fp16 (ACT for half, DVE for half). The software
            # DGE cast path tops out near 50GB/s of fp16 writes - far too slow
            # for the first half-panel, which gates the first matmul group.
            # Parity-based tile names: h and h+1 coexist; h+2's load waits
            # for h's last reader, which completes well before h+2's matmuls.
            chunks = []
            src = xT[:, h * bh:(h + 1) * bh].rearrange("(t p) b -> p t b", p=P)
            hc = cnt // 2
            for q in range(nxc):
                xc = xpool.tile([P, cnt, bh], f16, name=f"xc{h % 2}_{q}")
                for s in range(2):
                    j = 2 * q + s
                    xs = xspool.tile([P, hc, bh], f32, name=f"xs{j % 2}")
                    eng = nc.sync if j % 2 == 0 else nc.scalar
                    t0 = q * cnt + s * hc
                    eng.dma_start(
                        out=xs[:, :, :], in_=src[:, t0:t0 + hc, :])
                    dst = xc[:, s * hc:(s + 1) * hc, :]
                    if q < 2:
                        nc.scalar.copy(dst, xs[:, :, :])
                    else:
                        nc.vector.tensor_copy(dst, xs[:, :, :])
                chunks.append(xc)
            return chunks

        def phase1_half(kt, hk):
            """Dequantize half-k-tile (kt, hk) and XBAR-transpose into wdqT."""
            g0 = hk * HGR
            if hk == 0:
                nc.vector.tensor_scalar_mul(
                    sp16[:, kt, :], sc_t[:, kt, :], mu2_t[:, kt:kt + 1])
            # Raw int16 weight DMA. Flat [P, HN] staging keeps the DMA in
            # 4KB-contiguous runs; the grouped [P, HGR, GROUP] view is
            # AP-only (same contiguous bytes).
            wq_t = wqpool.tile([P, HN], i16, name="wq_t")
            nc.sync.dma_start(
                out=wq_t[:, :],
                in_=wq[kt * P:(kt + 1) * P, hk * HN:(hk + 1) * HN])
            sf = sfpool.tile([P, HGR, GROUP], f16, name="sf")
            nc.vector.tensor_tensor(
                sf[:, :, :],
                sp16[:, kt, g0:g0 + HGR].unsqueeze(-1).broadcast_to((P, HGR, GROUP)),
                mu1f[hk][:, :].rearrange("p (g r) -> p g r", r=GROUP),
                mul)
            wdq_t = wdqpool.tile([P, HGR, GROUP], f16, name="wdq_t")
            nc.vector.tensor_tensor(
                wdq_t[:, :, :],
                wq_t[:, :].rearrange("p (g r) -> p g r", r=GROUP),
                zr_t[:, kt, g0:g0 + HGR].unsqueeze(-1).broadcast_to((P, HGR, GROUP)),
                sub)
            nc.vector.tensor_tensor(wdq_t[:, :, :], wdq_t[:, :, :], sf[:, :, :], mul)
            # XBAR transpose [128 k, 2048 n] -> [(16 nt x 128 n), 128 k].
            nc.sync.dma_start(
                out=wdqT[kt][:, hk * (nnt // 2):(hk + 1) * (nnt // 2), :],
                in_=wdq_t[:, :, :],
                transpose=True)

        def phase1_ktile(kt):
            phase1_half(kt, 0)
            phase1_half(kt, 1)

        def matmuls(h, kt, xchunks):
            ps = opsum.tile([P, bh], f32, name="ops")
            for nt in range(nnt):
                nc.tensor.matmul(
                    ps[:, :],
                    lhsT=wdqT[kt][:, nt, :],
                    rhs=xchunks[nt // cnt][:, nt % cnt, :],
                    start=(nt == 0), stop=(nt == nnt - 1))
            ot = opool.tile([P, bh], f32, name="ot")
            nc.scalar.copy(ot[:, :], ps[:, :])
            nc.sync.dma_start(
                out=outT[kt * P:(kt + 1) * P, h * bh:(h + 1) * bh], in_=ot[:, :])

        # Interleave: the matmuls of BOTH h=0 and h=1 ride along with phase 1,
        # so the PE has ~13.6us of matmul work per k-tile while the dequant
        # pipeline (DVE-bound at ~11us/k-tile) produces the next weights.
        # phase1 keeps a 2-k-tile lead over the PE stream.
        phase1_ktile(0)
        phase1_ktile(1)
        xh0 = load_x_half(0)
        xh1 = load_x_half(1)
        for kt in range(nkt):
            if kt + 2 < nkt:
                phase1_ktile(kt + 2)
            matmuls(0, kt, xh0)
            matmuls(1, kt, xh1)
        for h in range(2, nh):
            xh = load_x_half(h)
            for kt in range(nkt):
                matmuls(h, kt, xh)

    # Run Bacc's compile passes (register allocation, sync-wait splitting
    # into EventSemaphores, nop fusion). The axon/PJRT exec path serializes
    # the module as-is, so finalize here.
    nc.finalize()
    return nc


def _get_program(key=()):
    if key not in _PROGRAM_CACHE:
        _PROGRAM_CACHE[key] = _build_program(*key) if key else _build_program()
    return _PROGRAM_CACHE[key]


def kernel(x, W_q, zeros, scales, mu1, mu2):
    global LAST_RESULTS
    from concourse.bass_utils import run_bass_kernel_spmd

    x = np.asarray(x)
    W_q = np.asarray(W_q)
    zeros = np.asarray(zeros)
    scales = np.asarray(scales)
    mu1 = np.asarray(mu1)
    mu2 = np.asarray(mu2)

    # Host-side layout prep (no arithmetic): transpose x, pad K to 8*1408,
    # pack the 0..15-valued W_q losslessly as int8.
    NKT = KC // P
    xT = np.ascontiguousarray(x.T)                      # [N, B] f32
    wq_p = np.zeros((KPAD, N), dtype=np.int16)
    wq_p[:K] = W_q.astype(np.int16)
    zr_p = np.zeros((KPAD, NG), dtype=zeros.dtype)
    zr_p[:K] = zeros.reshape(K, NG)
    sc_p = np.zeros((KPAD, NG), dtype=scales.dtype)
    sc_p[:K] = scales.reshape(K, NG)
    mu2_p = np.zeros((KPAD,), dtype=mu2.dtype)
    mu2_p[:K] = mu2

    def part_major(a2d):
        # [KC, G] -> [128, NKT*G], partition-major for a clean DMA
        g = a2d.shape[1]
        return np.ascontiguousarray(
            a2d.reshape(NKT, P, g).transpose(1, 0, 2).reshape(P, NKT * g))

    mu1_row = np.ascontiguousarray(mu1.reshape(1, N))
    in_maps = []
    for c in range(NCORES):
        lo, hi = c * KC, (c + 1) * KC
        in_maps.append({
            "xT": xT,
            "wq": np.ascontiguousarray(wq_p[lo:hi]),
            "zr": part_major(zr_p[lo:hi]),
            "sc": part_major(sc_p[lo:hi]),
            "mu1": mu1_row,
            "mu2": np.ascontiguousarray(mu2_p[lo:hi].reshape(NKT, P).T),
        })

    nc = _get_program()
    trace = bool(os.environ.get("KERNEL_TRACE"))
    res = run_bass_kernel_spmd(nc, in_maps, list(range(NCORES)), trace=trace)
    LAST_RESULTS = res

    out = np.empty((B, K), dtype=np.float32)
    for c in range(NCORES):
        lo = c * KC
        hi = min(lo + KC, K)
        out[:, lo:hi] = res.results[c]["outT"][:hi - lo].T
    return out
